# revision 10
# baseline (speedup 1.0000x reference)
"""TRN2 Bass kernel: fused multi-head attention (GPT-2 style, no causal mask).

Computes, for full inputs:
    qkv = X @ c_attn_w + c_attn_b ; q,k,v = split(qkv)
    per head: P = softmax(q k^T / sqrt(64)) ; a = P v
    out = merge_heads(a) @ c_proj_w + c_proj_b

Sharding: tensor-parallel over heads. 16 heads across 8 cores -> 2 heads/core.
Each core computes Q^T,K^T (d-major), V (token-major via PE transpose) for its
2 heads, attention in score-transposed orientation (S^T = K Q^T per block so the
softmax denominator comes out of the PE via an appended ones-column on V), and a
partial c_proj contribution (contraction over its 128 d-channels). Partials are
summed on the host (fp16 partials, fp32 host accumulate), which is the
tensor-parallel all-reduce.

All matmuls run in float32r (TF32-style PE mode, full rate at free-dim >= 256).
"""

import os
from contextlib import ExitStack

import numpy as np

import concourse.bass as bass
import concourse.mybir as mybir
from concourse import bacc, tile
from concourse.bass_utils import run_bass_kernel_spmd

F32 = mybir.dt.float32
F32R = mybir.dt.float32r
F16 = mybir.dt.float16

B, S, NX = 4, 1024, 1024
T = B * S  # 4096 tokens
NCORES = 8
HD = 64  # head dim
HPC = 2  # heads per core
V2S = 2 * (HD + 1)  # 130 columns per 128-token block in the V2 layout
EXP = mybir.ActivationFunctionType.Exp

_nc_cache = None


def _ensure_ntff_hook():
    """The agent image's `antenv` lacks `axon_hooks`, so bass_utils'
    trace=True path crashes on import. Synthesize the module and register
    the ctypes NTFF hook from trn_agent_boot (same thing boot() would have
    done). Returns True if profiling is available."""
    import sys
    import types

    try:
        from antenv.axon_hooks import get_axon_ntff_profile_hook  # noqa: F401

        return True
    except ImportError:
        pass
    try:
        import antenv
        from trn_agent_boot.trn_boot import _ntff_profile_via_ctypes

        mod = types.ModuleType("antenv.axon_hooks")
        mod._hook = _ntff_profile_via_ctypes("/opt/axon/libaxon_pjrt.so")

        def set_axon_ntff_profile_hook(h):
            mod._hook = h

        def get_axon_ntff_profile_hook():
            return mod._hook

        mod.set_axon_ntff_profile_hook = set_axon_ntff_profile_hook
        mod.get_axon_ntff_profile_hook = get_axon_ntff_profile_hook
        sys.modules["antenv.axon_hooks"] = mod
        antenv.axon_hooks = mod
        return True
    except Exception as e:  # pragma: no cover - profiling is best-effort
        print(f"kernel.py: NTFF profile hook unavailable ({e}); running untraced")
        return False


def _emit(nc, tc, xt, wq, wk, wv, wp, bq, bk, bv, identd, onecd, out):
    with ExitStack() as ctx:
        const = ctx.enter_context(tc.tile_pool(name="const", bufs=1))
        wq_sb = const.tile([128, 1024], F32R, tag="wq")
        wk_sb = const.tile([128, 1024], F32R, tag="wk")
        wv_sb = const.tile([128, 1024], F32R, tag="wv")
        wp_sb = const.tile([64, 2048], F32R, tag="wp")
        bq_sb = const.tile([128, 1], F32, tag="bq")
        bk_sb = const.tile([128, 1], F32, tag="bk")
        bv_sb = const.tile([128, 1], F32, tag="bv")
        ident = const.tile([128, 128], F32R, tag="ident")
        onec = const.tile([128, 64], F32R, tag="onec")
        qt = const.tile([128, T], F32R, tag="qt")
        kt = const.tile([128, T], F32R, tag="kt")
        vt = const.tile([128, T], F32R, tag="vt")
        v2 = const.tile([128, (T // 128) * V2S], F32R, tag="v2")
        at0 = const.tile([64, T], F32R, tag="at0")
        at1 = const.tile([64, T], F32R, tag="at1")

        nc.sync.dma_start(wq_sb[:], wq)
        nc.sync.dma_start(wk_sb[:], wk)
        nc.sync.dma_start(wv_sb[:], wv)
        nc.sync.dma_start(wp_sb[:], wp)
        nc.sync.dma_start(bq_sb[:], bq)
        nc.sync.dma_start(bk_sb[:], bk)
        nc.sync.dma_start(bv_sb[:], bv)
        nc.sync.dma_start(ident[:], identd)
        nc.sync.dma_start(onec[:], onecd)
        # ones columns of V2: per token block, col 64 (head 0) and col 129 (head 1)
        v2_ones = v2[:].rearrange("p (t h e) -> p t h e", h=2, e=HD + 1)[
            :, :, :, HD : HD + 1
        ]
        nc.vector.tensor_copy(
            v2_ones, onec[:].rearrange("p (t h e) -> p t h e", h=2, e=1)
        )

        # ---- Phase A: QKV projection (transposed: d-major) + V transposes ----
        with (
            tc.tile_pool(name="xtp", bufs=6) as xtp,
            tc.tile_pool(name="psA", bufs=2, space="PSUM") as psA,
        ):
            for qc in range(8):
                c0 = qc * 512
                psq = psA.tile([128, 512], F32, tag="psq")
                psk = psA.tile([128, 512], F32, tag="psk")
                psv = psA.tile([128, 512], F32, tag="psv")
                for k in range(8):
                    xt_t = xtp.tile([128, 512], F32R)
                    nc.sync.dma_start(
                        xt_t[:], xt[k * 128 : (k + 1) * 128, c0 : c0 + 512]
                    )
                    ks = slice(k * 128, (k + 1) * 128)
                    first, last = k == 0, k == 7
                    nc.tensor.matmul(
                        psq[:], wq_sb[:, ks], xt_t[:], start=first, stop=last
                    )
                    nc.tensor.matmul(
                        psk[:], wk_sb[:, ks], xt_t[:], start=first, stop=last
                    )
                    nc.tensor.matmul(
                        psv[:], wv_sb[:, ks], xt_t[:], start=first, stop=last
                    )
                nc.vector.tensor_scalar_add(qt[:, c0 : c0 + 512], psq[:], bq_sb[:, 0:1])
                nc.vector.tensor_scalar_add(kt[:, c0 : c0 + 512], psk[:], bk_sb[:, 0:1])
                nc.vector.tensor_scalar_add(vt[:, c0 : c0 + 512], psv[:], bv_sb[:, 0:1])
                # V^T -> V (token-major) through the PE, then strided copy into V2
                tp = psA.tile([128, 512], F32R, tag="tp")
                for t4 in range(4):
                    nc.tensor.transpose(
                        tp[:, t4 * 128 : (t4 + 1) * 128],
                        vt[:, c0 + t4 * 128 : c0 + (t4 + 1) * 128],
                        ident[:],
                    )
                src = tp[:].rearrange("p (t h e) -> p t h e", h=2, e=HD)
                dst = v2[:].rearrange("p (t h e) -> p t h e", h=2, e=HD + 1)[
                    :, qc * 4 : (qc + 1) * 4, :, 0:HD
                ]
                nc.vector.tensor_copy(dst, src)

        # ---- Phase B: attention per (batch, head, q-chunk); C: c_proj per batch ----
        with (
            tc.tile_pool(name="ptp", bufs=18) as ptp,
            tc.tile_pool(name="rcp", bufs=2) as rcp,
            tc.tile_pool(name="bcs", bufs=2) as bcs_pool,
            tc.tile_pool(name="obp", bufs=4) as obp,
            tc.tile_pool(name="st", bufs=2, space="PSUM") as st_pool,
            tc.tile_pool(name="ot", bufs=2, space="PSUM") as ot_pool,
            tc.tile_pool(name="bcp", bufs=2, space="PSUM") as bcp_pool,
            tc.tile_pool(name="opt", bufs=2, space="PSUM") as opt_pool,
        ):
            for b in range(4):
                for h in range(2):
                    hp = slice(h * 64, (h + 1) * 64)
                    for q2 in range(2):
                        q0 = b * 1024 + q2 * 512
                        pts = []
                        for kb in range(8):
                            k0 = b * 1024 + kb * 128
                            stt = st_pool.tile([128, 512], F32, tag="st")
                            # S^T block: keys on partitions, queries on free
                            nc.tensor.matmul(
                                stt[:],
                                kt[hp, k0 : k0 + 128],
                                qt[hp, q0 : q0 + 512],
                                start=True,
                                stop=True,
                            )
                            pt = ptp.tile([128, 512], F32R)
                            nc.scalar.activation(pt[:], stt[:], EXP, scale=0.125)
                            pts.append(pt)
                        ot = ot_pool.tile([65, 512], F32, tag="ot")
                        for kb in range(8):
                            tb = b * 8 + kb
                            v2c = tb * V2S + h * (HD + 1)
                            nc.tensor.matmul(
                                ot[:],
                                v2[:, v2c : v2c + HD + 1],
                                pts[kb][:],
                                start=(kb == 0),
                                stop=(kb == 7),
                            )
                        rc = rcp.tile([65, 512], F32R)
                        with nc.allow_low_precision(
                            reason="float32r has float32 bit layout"
                        ):
                            nc.vector.reciprocal(rc[64:65, :], ot[64:65, :])
                        # broadcast 1/rowsum across 64 partitions via the PE
                        # (ones[1,64].T @ rc[1,512]), then stage to SBUF so the
                        # normalize mul reads only one PSUM operand.
                        bcp = bcp_pool.tile([64, 512], F32, tag="bcp")
                        nc.tensor.matmul(
                            bcp[:],
                            onec[64:65, 0:64],
                            rc[64:65, :],
                            start=True,
                            stop=True,
                        )
                        bc = bcs_pool.tile([64, 512], F32R)
                        nc.vector.tensor_copy(bc[:], bcp[:])
                        at = at0 if h == 0 else at1
                        nc.vector.tensor_mul(at[:, q0 : q0 + 512], ot[0:64, :], bc[:])
                for q2 in range(2):
                    t0 = b * 1024 + q2 * 512
                    for oc in range(8):
                        opt = opt_pool.tile([128, 512], F32, tag="opt")
                        nc.tensor.matmul(
                            opt[:],
                            wp_sb[:, oc * 128 : (oc + 1) * 128],
                            at0[:, t0 : t0 + 512],
                            start=True,
                            stop=False,
                        )
                        nc.tensor.matmul(
                            opt[:],
                            wp_sb[:, 1024 + oc * 128 : 1024 + (oc + 1) * 128],
                            at1[:, t0 : t0 + 512],
                            start=False,
                            stop=True,
                        )
                        ob = obp.tile([128, 512], F16)
                        if oc % 2 == 0:
                            nc.vector.tensor_copy(ob[:], opt[:])
                        else:
                            nc.scalar.copy(ob[:], opt[:])
                        nc.sync.dma_start(
                            out[oc * 128 : (oc + 1) * 128, t0 : t0 + 512], ob[:]
                        )


def _build_nc():
    nc = bacc.Bacc(
        "TRN2",
        target_bir_lowering=False,
        debug=False,
        enable_asserts=False,
        num_devices=NCORES,
    )
    xt = nc.dram_tensor("xt", [NX, T], F32R, kind="ExternalInput").ap()
    wq = nc.dram_tensor("wq", [128, 1024], F32R, kind="ExternalInput").ap()
    wk = nc.dram_tensor("wk", [128, 1024], F32R, kind="ExternalInput").ap()
    wv = nc.dram_tensor("wv", [128, 1024], F32R, kind="ExternalInput").ap()
    wp = nc.dram_tensor("wp", [64, 2048], F32R, kind="ExternalInput").ap()
    bq = nc.dram_tensor("bq", [128, 1], F32, kind="ExternalInput").ap()
    bk = nc.dram_tensor("bk", [128, 1], F32, kind="ExternalInput").ap()
    bv = nc.dram_tensor("bv", [128, 1], F32, kind="ExternalInput").ap()
    identd = nc.dram_tensor("ident", [128, 128], F32R, kind="ExternalInput").ap()
    onecd = nc.dram_tensor("onec", [128, 64], F32R, kind="ExternalInput").ap()
    out = nc.dram_tensor("out_t", [NX, T], F16, kind="ExternalOutput").ap()
    with tile.TileContext(nc) as tc:
        _emit(nc, tc, xt, wq, wk, wv, wp, bq, bk, bv, identd, onecd, out)
    nc.compile()
    return nc


def _pack_w(wcols):
    # [1024, 128] -> [128, 8*128]: sbuf[p, k*128 + j] = W[k*128 + p, j]
    w = np.ascontiguousarray(np.asarray(wcols, dtype=np.float32))
    return np.ascontiguousarray(w.reshape(8, 128, 128).transpose(1, 0, 2).reshape(128, 1024))


def kernel(hidden_states, c_attn_w, c_attn_b, c_proj_w, c_proj_b):
    global _nc_cache
    hidden_states = np.asarray(hidden_states, dtype=np.float32)
    c_attn_w = np.asarray(c_attn_w, dtype=np.float32)
    c_attn_b = np.asarray(c_attn_b, dtype=np.float32)
    c_proj_w = np.asarray(c_proj_w, dtype=np.float32)
    c_proj_b = np.asarray(c_proj_b, dtype=np.float32)

    if _nc_cache is None:
        _nc_cache = _build_nc()
    nc = _nc_cache

    X = hidden_states.reshape(T, NX)
    xt_np = np.ascontiguousarray(X.T)

    in_maps = []
    for c in range(NCORES):
        cs = slice(c * 128, (c + 1) * 128)
        rows = c_proj_w[cs, :]
        wp_np = np.ascontiguousarray(
            np.concatenate([rows[0:64, :], rows[64:128, :]], axis=1)
        )
        in_maps.append(
            {
                "xt": xt_np,
                "wq": _pack_w(c_attn_w[:, c * 128 : (c + 1) * 128]),
                "wk": _pack_w(c_attn_w[:, 1024 + c * 128 : 1024 + (c + 1) * 128]),
                "wv": _pack_w(c_attn_w[:, 2048 + c * 128 : 2048 + (c + 1) * 128]),
                "wp": wp_np,
                "bq": np.ascontiguousarray(c_attn_b[cs].reshape(128, 1)),
                "bk": np.ascontiguousarray(c_attn_b[1024 + c * 128 : 1024 + (c + 1) * 128].reshape(128, 1)),
                "bv": np.ascontiguousarray(c_attn_b[2048 + c * 128 : 2048 + (c + 1) * 128].reshape(128, 1)),
                "ident": np.eye(128, dtype=np.float32),
                "onec": np.ones((128, 64), dtype=np.float32),
            }
        )

    trace = bool(int(os.environ.get("KERNEL_PROFILE", "0")))
    if trace:
        trace = _ensure_ntff_hook()
    try:
        res = run_bass_kernel_spmd(
            nc, in_maps, core_ids=list(range(NCORES)), trace=trace
        )
    except Exception:
        if not trace:
            raise
        print("kernel.py: traced run failed; retrying untraced")
        res = run_bass_kernel_spmd(nc, in_maps, core_ids=list(range(NCORES)))

    total = np.zeros((NX, T), np.float32)
    for r in res.results:
        total += r["out_t"].astype(np.float32)
    out = total.T.reshape(B, S, NX) + c_proj_b[None, None, :]
    kernel.last_exec_time_ns = res.exec_time_ns
    return out.astype(np.float32)


# revision 14
# speedup vs baseline: 1.2541x; 1.2541x over previous
"""TRN2 Bass kernel: fused multi-head attention (GPT-2 style, no causal mask).

Computes, for full inputs:
    qkv = X @ c_attn_w + c_attn_b ; q,k,v = split(qkv)
    per head: P = softmax(q k^T / sqrt(64)) ; a = P v
    out = merge_heads(a) @ c_proj_w + c_proj_b

Sharding: tensor-parallel over heads. 16 heads across 8 cores -> 2 heads/core.
Each core computes Q^T,K^T (d-major) and V (token-major, via PE transpose) for
its 2 heads, attention in score-transposed orientation (S^T = K Q^T per block,
so the softmax denominator falls out of the PE via an appended ones-column on
V), then a partial c_proj contribution (contraction over its 128 d-channels).
Partials are summed on the host (fp16 partials, fp32 host accumulate) - that
host sum is the tensor-parallel all-reduce.

Matmul operands are bf16 (fp32 PSUM accumulate); bf16 keeps the PE at its
~213ns/matmul N=512 cadence (fp32r self-loading weights measured 458ns). The
attention inner loop is software-pipelined: the P@V stage runs one (batch,
q-chunk) group behind the S^T/exp stage so TensorE, ScalarE and VectorE overlap.
"""

import os
from contextlib import ExitStack

import ml_dtypes
import numpy as np

import concourse.bass as bass
import concourse.mybir as mybir
from concourse import bacc, tile
from concourse.bass_utils import run_bass_kernel_spmd

F32 = mybir.dt.float32
F32R = mybir.dt.float32r
BF16 = mybir.dt.bfloat16
F16 = mybir.dt.float16

B, S, NX = 4, 1024, 1024
T = B * S  # 4096 tokens
NCORES = 8
HD = 64  # head dim
V2S = 2 * (HD + 1)  # 130 columns per 128-token block in the V2 layout
EXP = mybir.ActivationFunctionType.Exp
LN = mybir.ActivationFunctionType.Ln

_nc_cache = None


def _ensure_ntff_hook():
    """The agent image's `antenv` lacks `axon_hooks`, so bass_utils'
    trace=True path crashes on import. Synthesize the module and register
    the ctypes NTFF hook from trn_agent_boot (same thing boot() would have
    done). Returns True if profiling is available."""
    import sys
    import types

    try:
        from antenv.axon_hooks import get_axon_ntff_profile_hook  # noqa: F401

        return True
    except ImportError:
        pass
    try:
        import antenv
        from trn_agent_boot.trn_boot import _ntff_profile_via_ctypes

        mod = types.ModuleType("antenv.axon_hooks")
        mod._hook = _ntff_profile_via_ctypes("/opt/axon/libaxon_pjrt.so")

        def set_axon_ntff_profile_hook(h):
            mod._hook = h

        def get_axon_ntff_profile_hook():
            return mod._hook

        mod.set_axon_ntff_profile_hook = set_axon_ntff_profile_hook
        mod.get_axon_ntff_profile_hook = get_axon_ntff_profile_hook
        sys.modules["antenv.axon_hooks"] = mod
        antenv.axon_hooks = mod
        return True
    except Exception as e:  # pragma: no cover - profiling is best-effort
        print(f"kernel.py: NTFF profile hook unavailable ({e}); running untraced")
        return False


def _emit(nc, tc, xt, wq, wk, wv, wp, bq, bk, bv, identd, onecd, onefd, out):
    with ExitStack() as ctx:
        const = ctx.enter_context(tc.tile_pool(name="const", bufs=1))
        wq_sb = const.tile([128, 1024], BF16, tag="wq")
        wk_sb = const.tile([128, 1024], BF16, tag="wk")
        wv_sb = const.tile([128, 1024], BF16, tag="wv")
        wp_sb = const.tile([128, 1024], BF16, tag="wp")
        bq_sb = const.tile([128, 1], F32, tag="bq")
        bk_sb = const.tile([128, 1], F32, tag="bk")
        bv_sb = const.tile([128, 1], F32, tag="bv")
        ident = const.tile([128, 128], BF16, tag="ident")
        onec = const.tile([128, 64], BF16, tag="onec")
        onef = const.tile([128, 64], F32R, tag="onef")
        qt = const.tile([128, T], BF16, tag="qt")
        kt = const.tile([128, T], BF16, tag="kt")
        vt = const.tile([128, T], BF16, tag="vt")
        v2 = const.tile([128, (T // 128) * V2S], BF16, tag="v2")
        atall = const.tile([128, T], BF16, tag="atall")
        at1 = const.tile([64, T], BF16, tag="at1")

        nc.sync.dma_start(wq_sb[:], wq)
        nc.sync.dma_start(wk_sb[:], wk)
        nc.sync.dma_start(wv_sb[:], wv)
        nc.sync.dma_start(wp_sb[:], wp)
        nc.sync.dma_start(bq_sb[:], bq)
        nc.sync.dma_start(bk_sb[:], bk)
        nc.sync.dma_start(bv_sb[:], bv)
        nc.sync.dma_start(ident[:], identd)
        nc.sync.dma_start(onec[:], onecd)
        nc.sync.dma_start(onef[:], onefd)
        # ones columns of V2: per token block, col 64 (head 0) and col 129 (head 1)
        v2_ones = v2[:].rearrange("p (t h e) -> p t h e", h=2, e=HD + 1)[
            :, :, :, HD : HD + 1
        ]
        nc.vector.tensor_copy(
            v2_ones, onec[:].rearrange("p (t h e) -> p t h e", h=2, e=1)
        )

        # ---- Phase A: QKV projection (transposed: d-major) + V transposes ----
        with (
            tc.tile_pool(name="xtp", bufs=4) as xtp,
            tc.tile_pool(name="psA", bufs=2, space="PSUM") as psA,
        ):
            for qc in range(8):
                c0 = qc * 512
                xt_t = xtp.tile([128, 8 * 512], BF16)
                for k in range(8):
                    nc.sync.dma_start(
                        xt_t[:, k * 512 : (k + 1) * 512],
                        xt[k * 128 : (k + 1) * 128, c0 : c0 + 512],
                    )
                psq = psA.tile([128, 512], F32, tag="psq")
                psk = psA.tile([128, 512], F32, tag="psk")
                psv = psA.tile([128, 512], F32, tag="psv")
                for k in range(8):
                    ks = slice(k * 128, (k + 1) * 128)
                    rhs = xt_t[:, k * 512 : (k + 1) * 512]
                    first, last = k == 0, k == 7
                    nc.tensor.matmul(psq[:], wq_sb[:, ks], rhs, start=first, stop=last)
                    nc.tensor.matmul(psk[:], wk_sb[:, ks], rhs, start=first, stop=last)
                    nc.tensor.matmul(psv[:], wv_sb[:, ks], rhs, start=first, stop=last)
                nc.vector.tensor_scalar_add(qt[:, c0 : c0 + 512], psq[:], bq_sb[:, 0:1])
                nc.vector.tensor_scalar_add(kt[:, c0 : c0 + 512], psk[:], bk_sb[:, 0:1])
                nc.vector.tensor_scalar_add(vt[:, c0 : c0 + 512], psv[:], bv_sb[:, 0:1])
                # V^T -> V (token-major) through the PE, then strided copy into V2
                tp = psA.tile([128, 512], BF16, tag="tp")
                for t4 in range(4):
                    nc.tensor.transpose(
                        tp[:, t4 * 128 : (t4 + 1) * 128],
                        vt[:, c0 + t4 * 128 : c0 + (t4 + 1) * 128],
                        ident[:],
                    )
                src = tp[:].rearrange("p (t h e) -> p t h e", h=2, e=HD)
                dst = v2[:].rearrange("p (t h e) -> p t h e", h=2, e=HD + 1)[
                    :, qc * 4 : (qc + 1) * 4, :, 0:HD
                ]
                nc.vector.tensor_copy(dst, src)

        # ---- Phase B/C: software-pipelined attention + c_proj ----
        with (
            tc.tile_pool(name="ptp", bufs=36) as ptp,
            tc.tile_pool(name="rcp", bufs=2) as rcp,
            tc.tile_pool(name="bcs", bufs=2) as bcs_pool,
            tc.tile_pool(name="obp", bufs=3) as obp,
            tc.tile_pool(name="st", bufs=3, space="PSUM") as st_pool,
            tc.tile_pool(name="ot", bufs=2, space="PSUM") as ot_pool,
            tc.tile_pool(name="bcp", bufs=1, space="PSUM") as bcp_pool,
            tc.tile_pool(name="opt", bufs=2, space="PSUM") as opt_pool,
        ):

            def st_exp_stage(b, q2):
                """S^T blocks + exp for both heads of one (batch, q-chunk)."""
                q0 = b * 1024 + q2 * 512
                pts = {}
                for kb in range(8):
                    k0 = b * 1024 + kb * 128
                    for h in (0, 1):  # adjacent => concurrent PE row-groups
                        hp = slice(h * 64, (h + 1) * 64)
                        stt = st_pool.tile([128, 512], F32, tag="st")
                        nc.tensor.matmul(
                            stt[:],
                            kt[hp, k0 : k0 + 128],
                            qt[hp, q0 : q0 + 512],
                            start=True,
                            stop=True,
                        )
                        pt = ptp.tile([128, 512], BF16)
                        nc.scalar.activation(pt[:], stt[:], EXP, scale=0.125)
                        pts[(h, kb)] = pt
                return pts

            def ot_norm_stage(b, q2, pts):
                """P@V (+fused rowsum), normalize, write A^T for one group."""
                q0 = b * 1024 + q2 * 512
                for h in (0, 1):
                    ot = ot_pool.tile([65, 512], F32, tag="ot")
                    for kb in range(8):
                        v2c = (b * 8 + kb) * V2S + h * (HD + 1)
                        nc.tensor.matmul(
                            ot[:],
                            v2[:, v2c : v2c + HD + 1],
                            pts[(h, kb)][:],
                            start=(kb == 0),
                            stop=(kb == 7),
                        )
                    # 1/rowsum = exp(-ln(rowsum)) on ScalarE (standard ops
                    # only: custom DVE/GPSIMD ucode doesn't load in this
                    # runtime; nc.vector.reciprocal costs 3.4us per call)
                    ln = rcp.tile([65, 512], F32, tag="ln")
                    nc.scalar.activation(ln[64:65, :], ot[64:65, :], LN)
                    rc = rcp.tile([65, 512], F32R, tag="rc")
                    nc.scalar.activation(rc[64:65, :], ln[64:65, :], EXP, scale=-1.0)
                    bcp = bcp_pool.tile([64, 512], F32, tag="bcp")
                    nc.tensor.matmul(
                        bcp[:],
                        onef[64:65, 0:64],
                        rc[64:65, :],
                        start=True,
                        stop=True,
                    )
                    bc = bcs_pool.tile([64, 512], F32)
                    nc.vector.tensor_copy(bc[:], bcp[:])
                    if h == 0:
                        nc.vector.tensor_mul(
                            atall[0:64, q0 : q0 + 512], ot[0:64, :], bc[:]
                        )
                    else:
                        nc.vector.tensor_mul(
                            at1[0:64, q0 : q0 + 512], ot[0:64, :], bc[:]
                        )
                        # shift head-1 A^T onto partitions 64..127 so c_proj
                        # contracts all 128 d-channels in one matmul
                        nc.sync.dma_start(
                            atall[64:128, q0 : q0 + 512], at1[0:64, q0 : q0 + 512]
                        )

            def cproj_stage(b):
                for oc in range(8):
                    ob = obp.tile([128, 1024], F16)
                    for q2 in range(2):
                        t0 = b * 1024 + q2 * 512
                        opt = opt_pool.tile([128, 512], F32, tag="opt")
                        nc.tensor.matmul(
                            opt[:],
                            wp_sb[:, oc * 128 : (oc + 1) * 128],
                            atall[:, t0 : t0 + 512],
                            start=True,
                            stop=True,
                        )
                        if oc % 4 == 3:
                            nc.scalar.copy(ob[:, q2 * 512 : (q2 + 1) * 512], opt[:])
                        else:
                            nc.vector.tensor_copy(
                                ob[:, q2 * 512 : (q2 + 1) * 512], opt[:]
                            )
                    nc.sync.dma_start(
                        out[oc * 128 : (oc + 1) * 128, b * 1024 : (b + 1) * 1024],
                        ob[:],
                    )

            prev = None
            for b in range(4):
                for q2 in range(2):
                    if prev is not None:
                        pb, pq2, ppts = prev
                        ot_norm_stage(pb, pq2, ppts)
                        if pq2 == 1:
                            cproj_stage(pb)
                    prev = (b, q2, st_exp_stage(b, q2))
            pb, pq2, ppts = prev
            ot_norm_stage(pb, pq2, ppts)
            cproj_stage(pb)


def _build_nc():
    nc = bacc.Bacc(
        "TRN2",
        target_bir_lowering=False,
        debug=False,
        enable_asserts=False,
        num_devices=NCORES,
    )
    xt = nc.dram_tensor("xt", [NX, T], BF16, kind="ExternalInput").ap()
    wq = nc.dram_tensor("wq", [128, 1024], BF16, kind="ExternalInput").ap()
    wk = nc.dram_tensor("wk", [128, 1024], BF16, kind="ExternalInput").ap()
    wv = nc.dram_tensor("wv", [128, 1024], BF16, kind="ExternalInput").ap()
    wp = nc.dram_tensor("wp", [128, 1024], BF16, kind="ExternalInput").ap()
    bq = nc.dram_tensor("bq", [128, 1], F32, kind="ExternalInput").ap()
    bk = nc.dram_tensor("bk", [128, 1], F32, kind="ExternalInput").ap()
    bv = nc.dram_tensor("bv", [128, 1], F32, kind="ExternalInput").ap()
    identd = nc.dram_tensor("ident", [128, 128], BF16, kind="ExternalInput").ap()
    onecd = nc.dram_tensor("onec", [128, 64], BF16, kind="ExternalInput").ap()
    onefd = nc.dram_tensor("onef", [128, 64], F32R, kind="ExternalInput").ap()
    out = nc.dram_tensor("out_t", [NX, T], F16, kind="ExternalOutput").ap()
    with tile.TileContext(nc) as tc:
        _emit(nc, tc, xt, wq, wk, wv, wp, bq, bk, bv, identd, onecd, onefd, out)
    nc.compile()
    return nc


def _pack_w(wcols):
    # [1024, 128] -> [128, 8*128] bf16: sbuf[p, k*128 + j] = W[k*128 + p, j]
    w = np.ascontiguousarray(np.asarray(wcols, dtype=np.float32))
    return np.ascontiguousarray(
        w.reshape(8, 128, 128).transpose(1, 0, 2).reshape(128, 1024)
    ).astype(ml_dtypes.bfloat16)


def kernel(hidden_states, c_attn_w, c_attn_b, c_proj_w, c_proj_b):
    global _nc_cache
    hidden_states = np.asarray(hidden_states, dtype=np.float32)
    c_attn_w = np.asarray(c_attn_w, dtype=np.float32)
    c_attn_b = np.asarray(c_attn_b, dtype=np.float32)
    c_proj_w = np.asarray(c_proj_w, dtype=np.float32)
    c_proj_b = np.asarray(c_proj_b, dtype=np.float32)

    if _nc_cache is None:
        _nc_cache = _build_nc()
    nc = _nc_cache

    X = hidden_states.reshape(T, NX)
    xt_np = np.ascontiguousarray(X.T).astype(ml_dtypes.bfloat16)

    in_maps = []
    for c in range(NCORES):
        cs = slice(c * 128, (c + 1) * 128)
        in_maps.append(
            {
                "xt": xt_np,
                "wq": _pack_w(c_attn_w[:, c * 128 : (c + 1) * 128]),
                "wk": _pack_w(c_attn_w[:, 1024 + c * 128 : 1024 + (c + 1) * 128]),
                "wv": _pack_w(c_attn_w[:, 2048 + c * 128 : 2048 + (c + 1) * 128]),
                "wp": np.ascontiguousarray(c_proj_w[cs, :]).astype(ml_dtypes.bfloat16),
                "bq": np.ascontiguousarray(c_attn_b[cs].reshape(128, 1)),
                "bk": np.ascontiguousarray(
                    c_attn_b[1024 + c * 128 : 1024 + (c + 1) * 128].reshape(128, 1)
                ),
                "bv": np.ascontiguousarray(
                    c_attn_b[2048 + c * 128 : 2048 + (c + 1) * 128].reshape(128, 1)
                ),
                "ident": np.eye(128, dtype=np.float32).astype(ml_dtypes.bfloat16),
                "onec": np.ones((128, 64), dtype=ml_dtypes.bfloat16),
                "onef": np.ones((128, 64), dtype=np.float32),
            }
        )

    trace = bool(int(os.environ.get("KERNEL_PROFILE", "0")))
    if trace:
        trace = _ensure_ntff_hook()
    try:
        res = run_bass_kernel_spmd(
            nc, in_maps, core_ids=list(range(NCORES)), trace=trace
        )
    except Exception:
        if not trace:
            raise
        print("kernel.py: traced run failed; retrying untraced")
        res = run_bass_kernel_spmd(nc, in_maps, core_ids=list(range(NCORES)))

    total = np.zeros((NX, T), np.float32)
    for r in res.results:
        total += r["out_t"].astype(np.float32)
    out = total.T.reshape(B, S, NX) + c_proj_b[None, None, :]
    kernel.last_exec_time_ns = res.exec_time_ns
    return out.astype(np.float32)


# revision 15
# speedup vs baseline: 1.3941x; 1.1117x over previous
"""TRN2 Bass kernel: fused multi-head attention (GPT-2 style, no causal mask).

Computes, for full inputs:
    qkv = X @ c_attn_w + c_attn_b ; q,k,v = split(qkv)
    per head: P = softmax(q k^T / sqrt(64)) ; a = P v
    out = merge_heads(a) @ c_proj_w + c_proj_b

Sharding: tensor-parallel over heads. 16 heads across 8 cores -> 2 heads/core.
Each core computes Q^T,K^T (d-major) and V (token-major, via PE transpose) for
its 2 heads, attention in score-transposed orientation (S^T = K Q^T per block,
so the softmax denominator falls out of the PE via an appended ones-column on
V), then a partial c_proj contribution (contraction over its 128 d-channels).
Partials are summed on the host (fp16 partials, fp32 host accumulate) - that
host sum is the tensor-parallel all-reduce.

Matmul operands are bf16 (fp32 PSUM accumulate); bf16 keeps the PE at its
~213ns/matmul N=512 cadence (fp32r self-loading weights measured 458ns). The
attention inner loop is software-pipelined: the P@V stage runs one (batch,
q-chunk) group behind the S^T/exp stage so TensorE, ScalarE and VectorE overlap.
"""

import os
from contextlib import ExitStack

import ml_dtypes
import numpy as np

import concourse.bass as bass
import concourse.mybir as mybir
from concourse import bacc, tile
from concourse.bass_utils import run_bass_kernel_spmd

F32 = mybir.dt.float32
F32R = mybir.dt.float32r
BF16 = mybir.dt.bfloat16
F16 = mybir.dt.float16
I32 = mybir.dt.int32

B, S, NX = 4, 1024, 1024
T = B * S  # 4096 tokens
NCORES = 8
HD = 64  # head dim
V2S = 2 * (HD + 1)  # 130 columns per 128-token block in the V2 layout
EXP = mybir.ActivationFunctionType.Exp
RECIP_MAGIC = 0x7EF311C3  # fp32 bit-trick reciprocal seed, ~5% rel err

_nc_cache = None


def _ensure_ntff_hook():
    """The agent image's `antenv` lacks `axon_hooks`, so bass_utils'
    trace=True path crashes on import. Synthesize the module and register
    the ctypes NTFF hook from trn_agent_boot (same thing boot() would have
    done). Returns True if profiling is available."""
    import sys
    import types

    try:
        from antenv.axon_hooks import get_axon_ntff_profile_hook  # noqa: F401

        return True
    except ImportError:
        pass
    try:
        import antenv
        from trn_agent_boot.trn_boot import _ntff_profile_via_ctypes

        mod = types.ModuleType("antenv.axon_hooks")
        mod._hook = _ntff_profile_via_ctypes("/opt/axon/libaxon_pjrt.so")

        def set_axon_ntff_profile_hook(h):
            mod._hook = h

        def get_axon_ntff_profile_hook():
            return mod._hook

        mod.set_axon_ntff_profile_hook = set_axon_ntff_profile_hook
        mod.get_axon_ntff_profile_hook = get_axon_ntff_profile_hook
        sys.modules["antenv.axon_hooks"] = mod
        antenv.axon_hooks = mod
        return True
    except Exception as e:  # pragma: no cover - profiling is best-effort
        print(f"kernel.py: NTFF profile hook unavailable ({e}); running untraced")
        return False


def _emit(nc, tc, xt, wq, wk, wv, wp, bq, bk, bv, identd, onecd, onefd, out):
    with ExitStack() as ctx:
        const = ctx.enter_context(tc.tile_pool(name="const", bufs=1))
        wq_sb = const.tile([128, 1024], BF16, tag="wq")
        wk_sb = const.tile([128, 1024], BF16, tag="wk")
        wv_sb = const.tile([128, 1024], BF16, tag="wv")
        wp_sb = const.tile([128, 1024], BF16, tag="wp")
        bq_sb = const.tile([128, 1], F32, tag="bq")
        bk_sb = const.tile([128, 1], F32, tag="bk")
        bv_sb = const.tile([128, 1], F32, tag="bv")
        ident = const.tile([128, 128], BF16, tag="ident")
        onec = const.tile([128, 64], BF16, tag="onec")
        onef = const.tile([128, 64], F32R, tag="onef")
        qt = const.tile([128, T], BF16, tag="qt")
        kt = const.tile([128, T], BF16, tag="kt")
        vt = const.tile([128, T], BF16, tag="vt")
        v2 = const.tile([128, (T // 128) * V2S], BF16, tag="v2")
        atall = const.tile([128, T], BF16, tag="atall")
        at1 = const.tile([64, T], BF16, tag="at1")

        nc.sync.dma_start(wq_sb[:], wq)
        nc.sync.dma_start(wk_sb[:], wk)
        nc.sync.dma_start(wv_sb[:], wv)
        nc.sync.dma_start(wp_sb[:], wp)
        nc.sync.dma_start(bq_sb[:], bq)
        nc.sync.dma_start(bk_sb[:], bk)
        nc.sync.dma_start(bv_sb[:], bv)
        nc.sync.dma_start(ident[:], identd)
        nc.sync.dma_start(onec[:], onecd)
        nc.sync.dma_start(onef[:], onefd)
        # ones columns of V2: per token block, col 64 (head 0) and col 129 (head 1)
        v2_ones = v2[:].rearrange("p (t h e) -> p t h e", h=2, e=HD + 1)[
            :, :, :, HD : HD + 1
        ]
        nc.vector.tensor_copy(
            v2_ones, onec[:].rearrange("p (t h e) -> p t h e", h=2, e=1)
        )

        # ---- Phase A: QKV projection (transposed: d-major) + V transposes ----
        with (
            tc.tile_pool(name="xtp", bufs=4) as xtp,
            tc.tile_pool(name="psA", bufs=2, space="PSUM") as psA,
        ):
            for qc in range(8):
                c0 = qc * 512
                xt_t = xtp.tile([128, 8 * 512], BF16)
                for k in range(8):
                    nc.sync.dma_start(
                        xt_t[:, k * 512 : (k + 1) * 512],
                        xt[k * 128 : (k + 1) * 128, c0 : c0 + 512],
                    )
                psq = psA.tile([128, 512], F32, tag="psq")
                psk = psA.tile([128, 512], F32, tag="psk")
                psv = psA.tile([128, 512], F32, tag="psv")
                for k in range(8):
                    ks = slice(k * 128, (k + 1) * 128)
                    rhs = xt_t[:, k * 512 : (k + 1) * 512]
                    first, last = k == 0, k == 7
                    nc.tensor.matmul(psq[:], wq_sb[:, ks], rhs, start=first, stop=last)
                    nc.tensor.matmul(psk[:], wk_sb[:, ks], rhs, start=first, stop=last)
                    nc.tensor.matmul(psv[:], wv_sb[:, ks], rhs, start=first, stop=last)
                nc.vector.tensor_scalar_add(qt[:, c0 : c0 + 512], psq[:], bq_sb[:, 0:1])
                nc.vector.tensor_scalar_add(kt[:, c0 : c0 + 512], psk[:], bk_sb[:, 0:1])
                nc.vector.tensor_scalar_add(vt[:, c0 : c0 + 512], psv[:], bv_sb[:, 0:1])
                # V^T -> V (token-major) through the PE, then strided copy into V2
                tp = psA.tile([128, 512], BF16, tag="tp")
                for t4 in range(4):
                    nc.tensor.transpose(
                        tp[:, t4 * 128 : (t4 + 1) * 128],
                        vt[:, c0 + t4 * 128 : c0 + (t4 + 1) * 128],
                        ident[:],
                    )
                src = tp[:].rearrange("p (t h e) -> p t h e", h=2, e=HD)
                dst = v2[:].rearrange("p (t h e) -> p t h e", h=2, e=HD + 1)[
                    :, qc * 4 : (qc + 1) * 4, :, 0:HD
                ]
                nc.vector.tensor_copy(dst, src)

        # ---- Phase B/C: software-pipelined attention + c_proj ----
        # 3-deep pipeline per (batch, q-chunk) group g:
        #   S1(g): S^T matmuls + exp          (PE + ACT)
        #   S2(g): P@V matmuls + 1/rowsum on DVE (bit-trick seed + 2 Newton)
        #   S3(g): broadcast matmul + normalize (+ c_proj at batch end)
        # S3 runs two groups behind S1 so the PE never waits on the DVE
        # reciprocal chain (engines issue strictly in order).
        with (
            tc.tile_pool(name="ptp", bufs=36) as ptp,
            tc.tile_pool(name="rcp", bufs=3) as rcp,
            tc.tile_pool(name="bcs", bufs=2) as bcs_pool,
            tc.tile_pool(name="obp", bufs=3) as obp,
            tc.tile_pool(name="st", bufs=2, space="PSUM") as st_pool,
            tc.tile_pool(name="ot", bufs=4, space="PSUM") as ot_pool,
            tc.tile_pool(name="po", bufs=2, space="PSUM") as po_pool,
        ):

            def s1_st_exp(b, q2):
                q0 = b * 1024 + q2 * 512
                pts = {}
                for kb in range(8):
                    k0 = b * 1024 + kb * 128
                    for h in (0, 1):  # adjacent => concurrent PE row-groups
                        hp = slice(h * 64, (h + 1) * 64)
                        stt = st_pool.tile([128, 512], F32, tag="st")
                        nc.tensor.matmul(
                            stt[:],
                            kt[hp, k0 : k0 + 128],
                            qt[hp, q0 : q0 + 512],
                            start=True,
                            stop=True,
                        )
                        pt = ptp.tile([128, 512], BF16)
                        nc.scalar.activation(pt[:], stt[:], EXP, scale=0.125)
                        pts[(h, kb)] = pt
                return pts

            def s2_ot_recip(b, q2, pts):
                state = {}
                for h in (0, 1):
                    ot = ot_pool.tile([65, 512], F32, tag="ot")
                    for kb in range(8):
                        v2c = (b * 8 + kb) * V2S + h * (HD + 1)
                        nc.tensor.matmul(
                            ot[:],
                            v2[:, v2c : v2c + HD + 1],
                            pts[(h, kb)][:],
                            start=(kb == 0),
                            stop=(kb == 7),
                        )
                    # 1/rowsum on DVE only (no ACT table funcs, no custom ops):
                    # y0 = bitcast(MAGIC - bits(s)); two Newton steps with the
                    # sign carried by the minus-ones broadcast weights.
                    sd = rcp.tile([65, 512], I32, tag="sd")
                    nc.vector.tensor_scalar(
                        sd[64:65, :],
                        ot[64:65, :].bitcast(I32),
                        -1,
                        RECIP_MAGIC,
                        mybir.AluOpType.mult,
                        mybir.AluOpType.add,
                    )
                    y0 = sd[64:65, :].bitcast(F32)
                    t1 = rcp.tile([65, 512], F32, tag="t1")
                    nc.vector.tensor_mul(t1[64:65, :], ot[64:65, :], y0)
                    m1 = rcp.tile([65, 512], F32, tag="m1")  # m1 = -y1
                    nc.vector.scalar_tensor_tensor(
                        m1[64:65, :],
                        t1[64:65, :],
                        2.0,
                        y0,
                        mybir.AluOpType.subtract,
                        mybir.AluOpType.mult,
                    )
                    t2 = rcp.tile([65, 512], F32, tag="t2")  # t2 = -s*y1
                    nc.vector.tensor_mul(t2[64:65, :], ot[64:65, :], m1[64:65, :])
                    m2 = rcp.tile([65, 512], F32R, tag="m2")  # m2 = -y2
                    nc.vector.scalar_tensor_tensor(
                        m2[64:65, :],
                        t2[64:65, :],
                        2.0,
                        m1[64:65, :],
                        mybir.AluOpType.add,
                        mybir.AluOpType.mult,
                    )
                    state[h] = (ot, m2)
                return state

            def s3_norm(b, q2, state):
                q0 = b * 1024 + q2 * 512
                for h in (0, 1):
                    ot, m2 = state[h]
                    bcp = po_pool.tile([64, 512], F32, tag="po")
                    nc.tensor.matmul(
                        bcp[:],
                        onef[64:65, 0:64],
                        m2[64:65, :],
                        start=True,
                        stop=True,
                    )
                    bc = bcs_pool.tile([64, 512], F32)
                    nc.scalar.copy(bc[:], bcp[:])
                    at = atall if h == 0 else at1
                    nc.vector.tensor_mul(at[0:64, q0 : q0 + 512], ot[0:64, :], bc[:])
                    if h == 1:
                        nc.sync.dma_start(
                            atall[64:128, q0 : q0 + 512], at1[0:64, q0 : q0 + 512]
                        )

            def cproj_stage(b):
                for oc in range(8):
                    ob = obp.tile([128, 1024], F16)
                    for q2 in range(2):
                        t0 = b * 1024 + q2 * 512
                        opt = po_pool.tile([128, 512], F32, tag="po")
                        nc.tensor.matmul(
                            opt[:],
                            wp_sb[:, oc * 128 : (oc + 1) * 128],
                            atall[:, t0 : t0 + 512],
                            start=True,
                            stop=True,
                        )
                        if oc % 4 == 3:
                            nc.scalar.copy(ob[:, q2 * 512 : (q2 + 1) * 512], opt[:])
                        else:
                            nc.vector.tensor_copy(
                                ob[:, q2 * 512 : (q2 + 1) * 512], opt[:]
                            )
                    nc.sync.dma_start(
                        out[oc * 128 : (oc + 1) * 128, b * 1024 : (b + 1) * 1024],
                        ob[:],
                    )

            groups = [(b, q2) for b in range(4) for q2 in range(2)]
            live = []  # (b, q2, pts_or_state, stage)
            s1_out = {}
            s2_out = {}
            for i, (b, q2) in enumerate(groups):
                if i >= 1:
                    pb, pq2 = groups[i - 1]
                    s2_out[(pb, pq2)] = s2_ot_recip(pb, pq2, s1_out.pop((pb, pq2)))
                if i >= 2:
                    qb, qq2 = groups[i - 2]
                    s3_norm(qb, qq2, s2_out.pop((qb, qq2)))
                    if qq2 == 1:
                        cproj_stage(qb)
                s1_out[(b, q2)] = s1_st_exp(b, q2)
            # drain
            b, q2 = groups[-1]
            s2_out[(b, q2)] = s2_ot_recip(b, q2, s1_out.pop((b, q2)))
            for (qb, qq2) in groups[-2:]:
                s3_norm(qb, qq2, s2_out.pop((qb, qq2)))
                if qq2 == 1:
                    cproj_stage(qb)


def _build_nc():
    nc = bacc.Bacc(
        "TRN2",
        target_bir_lowering=False,
        debug=False,
        enable_asserts=False,
        num_devices=NCORES,
    )
    xt = nc.dram_tensor("xt", [NX, T], BF16, kind="ExternalInput").ap()
    wq = nc.dram_tensor("wq", [128, 1024], BF16, kind="ExternalInput").ap()
    wk = nc.dram_tensor("wk", [128, 1024], BF16, kind="ExternalInput").ap()
    wv = nc.dram_tensor("wv", [128, 1024], BF16, kind="ExternalInput").ap()
    wp = nc.dram_tensor("wp", [128, 1024], BF16, kind="ExternalInput").ap()
    bq = nc.dram_tensor("bq", [128, 1], F32, kind="ExternalInput").ap()
    bk = nc.dram_tensor("bk", [128, 1], F32, kind="ExternalInput").ap()
    bv = nc.dram_tensor("bv", [128, 1], F32, kind="ExternalInput").ap()
    identd = nc.dram_tensor("ident", [128, 128], BF16, kind="ExternalInput").ap()
    onecd = nc.dram_tensor("onec", [128, 64], BF16, kind="ExternalInput").ap()
    onefd = nc.dram_tensor("onef", [128, 64], F32R, kind="ExternalInput").ap()
    out = nc.dram_tensor("out_t", [NX, T], F16, kind="ExternalOutput").ap()
    with tile.TileContext(nc) as tc:
        _emit(nc, tc, xt, wq, wk, wv, wp, bq, bk, bv, identd, onecd, onefd, out)
    nc.compile()
    return nc


def _pack_w(wcols):
    # [1024, 128] -> [128, 8*128] bf16: sbuf[p, k*128 + j] = W[k*128 + p, j]
    w = np.ascontiguousarray(np.asarray(wcols, dtype=np.float32))
    return np.ascontiguousarray(
        w.reshape(8, 128, 128).transpose(1, 0, 2).reshape(128, 1024)
    ).astype(ml_dtypes.bfloat16)


def kernel(hidden_states, c_attn_w, c_attn_b, c_proj_w, c_proj_b):
    global _nc_cache
    hidden_states = np.asarray(hidden_states, dtype=np.float32)
    c_attn_w = np.asarray(c_attn_w, dtype=np.float32)
    c_attn_b = np.asarray(c_attn_b, dtype=np.float32)
    c_proj_w = np.asarray(c_proj_w, dtype=np.float32)
    c_proj_b = np.asarray(c_proj_b, dtype=np.float32)

    if _nc_cache is None:
        _nc_cache = _build_nc()
    nc = _nc_cache

    X = hidden_states.reshape(T, NX)
    xt_np = np.ascontiguousarray(X.T).astype(ml_dtypes.bfloat16)

    in_maps = []
    for c in range(NCORES):
        cs = slice(c * 128, (c + 1) * 128)
        in_maps.append(
            {
                "xt": xt_np,
                "wq": _pack_w(c_attn_w[:, c * 128 : (c + 1) * 128]),
                "wk": _pack_w(c_attn_w[:, 1024 + c * 128 : 1024 + (c + 1) * 128]),
                "wv": _pack_w(c_attn_w[:, 2048 + c * 128 : 2048 + (c + 1) * 128]),
                "wp": np.ascontiguousarray(c_proj_w[cs, :]).astype(ml_dtypes.bfloat16),
                "bq": np.ascontiguousarray(c_attn_b[cs].reshape(128, 1)),
                "bk": np.ascontiguousarray(
                    c_attn_b[1024 + c * 128 : 1024 + (c + 1) * 128].reshape(128, 1)
                ),
                "bv": np.ascontiguousarray(
                    c_attn_b[2048 + c * 128 : 2048 + (c + 1) * 128].reshape(128, 1)
                ),
                "ident": np.eye(128, dtype=np.float32).astype(ml_dtypes.bfloat16),
                "onec": np.ones((128, 64), dtype=ml_dtypes.bfloat16),
                "onef": np.full((128, 64), -1.0, dtype=np.float32),
            }
        )

    trace = bool(int(os.environ.get("KERNEL_PROFILE", "0")))
    if trace:
        trace = _ensure_ntff_hook()
    try:
        res = run_bass_kernel_spmd(
            nc, in_maps, core_ids=list(range(NCORES)), trace=trace
        )
    except Exception:
        if not trace:
            raise
        print("kernel.py: traced run failed; retrying untraced")
        res = run_bass_kernel_spmd(nc, in_maps, core_ids=list(range(NCORES)))

    total = np.zeros((NX, T), np.float32)
    for r in res.results:
        total += r["out_t"].astype(np.float32)
    out = total.T.reshape(B, S, NX) + c_proj_b[None, None, :]
    kernel.last_exec_time_ns = res.exec_time_ns
    return out.astype(np.float32)


# revision 17
# speedup vs baseline: 1.4296x; 1.0254x over previous
"""TRN2 Bass kernel: fused multi-head attention (GPT-2 style, no causal mask).

Computes, for full inputs:
    qkv = X @ c_attn_w + c_attn_b ; q,k,v = split(qkv)
    per head: P = softmax(q k^T / sqrt(64)) ; a = P v
    out = merge_heads(a) @ c_proj_w + c_proj_b

Sharding: tensor-parallel over heads. 16 heads across 8 cores -> 2 heads/core.
Each core computes Q^T,K^T (d-major) and V (token-major, via PE transpose) for
its 2 heads, attention in score-transposed orientation (S^T = K Q^T per block,
so the softmax denominator falls out of the PE via an appended ones-column on
V), then a partial c_proj contribution (contraction over its 128 d-channels).
Partials are summed on the host (fp16 partials, fp32 host accumulate) - that
host sum is the tensor-parallel all-reduce.

Matmul operands are bf16 (fp32 PSUM accumulate); bf16 keeps the PE at its
~213ns/matmul N=512 cadence (fp32r self-loading weights measured 458ns). The
attention inner loop is software-pipelined: the P@V stage runs one (batch,
q-chunk) group behind the S^T/exp stage so TensorE, ScalarE and VectorE overlap.
"""

import os
from contextlib import ExitStack

import ml_dtypes
import numpy as np

import concourse.bass as bass
import concourse.mybir as mybir
from concourse import bacc, tile
from concourse.bass_utils import run_bass_kernel_spmd

F32 = mybir.dt.float32
F32R = mybir.dt.float32r
BF16 = mybir.dt.bfloat16
F16 = mybir.dt.float16
I32 = mybir.dt.int32

B, S, NX = 4, 1024, 1024
T = B * S  # 4096 tokens
NCORES = 8
HD = 64  # head dim
V2S = 2 * (HD + 1)  # 130 columns per 128-token block in the V2 layout
EXP = mybir.ActivationFunctionType.Exp
RECIP_MAGIC = 0x7EF311C3  # fp32 bit-trick reciprocal seed, ~5% rel err

_nc_cache = None


def _ensure_ntff_hook():
    """The agent image's `antenv` lacks `axon_hooks`, so bass_utils'
    trace=True path crashes on import. Synthesize the module and register
    the ctypes NTFF hook from trn_agent_boot (same thing boot() would have
    done). Returns True if profiling is available."""
    import sys
    import types

    try:
        from antenv.axon_hooks import get_axon_ntff_profile_hook  # noqa: F401

        return True
    except ImportError:
        pass
    try:
        import antenv
        from trn_agent_boot.trn_boot import _ntff_profile_via_ctypes

        mod = types.ModuleType("antenv.axon_hooks")
        mod._hook = _ntff_profile_via_ctypes("/opt/axon/libaxon_pjrt.so")

        def set_axon_ntff_profile_hook(h):
            mod._hook = h

        def get_axon_ntff_profile_hook():
            return mod._hook

        mod.set_axon_ntff_profile_hook = set_axon_ntff_profile_hook
        mod.get_axon_ntff_profile_hook = get_axon_ntff_profile_hook
        sys.modules["antenv.axon_hooks"] = mod
        antenv.axon_hooks = mod
        return True
    except Exception as e:  # pragma: no cover - profiling is best-effort
        print(f"kernel.py: NTFF profile hook unavailable ({e}); running untraced")
        return False


def _emit(nc, tc, xt, wq, wk, wv, wp, bq, bk, bv, identd, onecd, onefd, out):
    with ExitStack() as ctx:
        const = ctx.enter_context(tc.tile_pool(name="const", bufs=1))
        wq_sb = const.tile([128, 1024], BF16, tag="wq")
        wk_sb = const.tile([128, 1024], BF16, tag="wk")
        wv_sb = const.tile([128, 1024], BF16, tag="wv")
        wp_sb = const.tile([128, 1024], BF16, tag="wp")
        bq_sb = const.tile([128, 1], F32, tag="bq")
        bk_sb = const.tile([128, 1], F32, tag="bk")
        bv_sb = const.tile([128, 1], F32, tag="bv")
        ident = const.tile([128, 128], BF16, tag="ident")
        onec = const.tile([128, 64], BF16, tag="onec")
        onef = const.tile([128, 64], F32R, tag="onef")
        qt = const.tile([128, T], BF16, tag="qt")
        kt = const.tile([128, T], BF16, tag="kt")
        vt = const.tile([128, T], BF16, tag="vt")
        v2 = const.tile([128, (T // 128) * V2S], BF16, tag="v2")
        atall = const.tile([128, T], BF16, tag="atall")
        at1 = const.tile([64, T], BF16, tag="at1")

        nc.sync.dma_start(wq_sb[:], wq)
        nc.sync.dma_start(wk_sb[:], wk)
        nc.sync.dma_start(wv_sb[:], wv)
        nc.sync.dma_start(wp_sb[:], wp)
        nc.sync.dma_start(bq_sb[:], bq)
        nc.sync.dma_start(bk_sb[:], bk)
        nc.sync.dma_start(bv_sb[:], bv)
        nc.sync.dma_start(ident[:], identd)
        nc.sync.dma_start(onec[:], onecd)
        nc.sync.dma_start(onef[:], onefd)
        # ones columns of V2: per token block, col 64 (head 0) and col 129 (head 1)
        v2_ones = v2[:].rearrange("p (t h e) -> p t h e", h=2, e=HD + 1)[
            :, :, :, HD : HD + 1
        ]
        nc.vector.tensor_copy(
            v2_ones, onec[:].rearrange("p (t h e) -> p t h e", h=2, e=1)
        )

        # ---- Phase A: QKV projection (transposed: d-major) + V transposes ----
        with (
            tc.tile_pool(name="xtp", bufs=4) as xtp,
            tc.tile_pool(name="psA", bufs=2, space="PSUM") as psA,
        ):
            for qc in range(8):
                c0 = qc * 512
                xt_t = xtp.tile([128, 8 * 512], BF16)
                for k in range(8):
                    nc.sync.dma_start(
                        xt_t[:, k * 512 : (k + 1) * 512],
                        xt[k * 128 : (k + 1) * 128, c0 : c0 + 512],
                    )
                psq = psA.tile([128, 512], F32, tag="psq")
                psk = psA.tile([128, 512], F32, tag="psk")
                psv = psA.tile([128, 512], F32, tag="psv")
                for k in range(8):
                    ks = slice(k * 128, (k + 1) * 128)
                    rhs = xt_t[:, k * 512 : (k + 1) * 512]
                    first, last = k == 0, k == 7
                    nc.tensor.matmul(psq[:], wq_sb[:, ks], rhs, start=first, stop=last)
                    nc.tensor.matmul(psk[:], wk_sb[:, ks], rhs, start=first, stop=last)
                    nc.tensor.matmul(psv[:], wv_sb[:, ks], rhs, start=first, stop=last)
                nc.vector.tensor_scalar_add(qt[:, c0 : c0 + 512], psq[:], bq_sb[:, 0:1])
                nc.vector.tensor_scalar_add(kt[:, c0 : c0 + 512], psk[:], bk_sb[:, 0:1])
                nc.vector.tensor_scalar_add(vt[:, c0 : c0 + 512], psv[:], bv_sb[:, 0:1])
                # V^T -> V (token-major) through the PE, then strided copy into V2
                tp = psA.tile([128, 512], BF16, tag="tp")
                for t4 in range(4):
                    nc.tensor.transpose(
                        tp[:, t4 * 128 : (t4 + 1) * 128],
                        vt[:, c0 + t4 * 128 : c0 + (t4 + 1) * 128],
                        ident[:],
                    )
                src = tp[:].rearrange("p (t h e) -> p t h e", h=2, e=HD)
                dst = v2[:].rearrange("p (t h e) -> p t h e", h=2, e=HD + 1)[
                    :, qc * 4 : (qc + 1) * 4, :, 0:HD
                ]
                nc.vector.tensor_copy(dst, src)

        # ---- Phase B/C: software-pipelined attention + c_proj ----
        # 3-deep pipeline per (batch, q-chunk) group g:
        #   S1(g): S^T matmuls + exp          (PE + ACT)
        #   S2(g): P@V matmuls + 1/rowsum on DVE (bit-trick seed + 2 Newton)
        #   S3(g): broadcast matmul + normalize (+ c_proj at batch end)
        # S3 runs two groups behind S1 so the PE never waits on the DVE
        # reciprocal chain (engines issue strictly in order).
        with (
            tc.tile_pool(name="ptp", bufs=36) as ptp,
            tc.tile_pool(name="rcp", bufs=3) as rcp,
            tc.tile_pool(name="bcs", bufs=2) as bcs_pool,
            tc.tile_pool(name="obp", bufs=3) as obp,
            tc.tile_pool(name="st", bufs=2, space="PSUM") as st_pool,
            tc.tile_pool(name="ot", bufs=4, space="PSUM") as ot_pool,
            tc.tile_pool(name="po", bufs=2, space="PSUM") as po_pool,
        ):

            def s1s2_interleaved(b, q2, prev):
                # emit OT chains of the previous group between this group's
                # isolated S^T matmuls so PE drains/fills overlap
                q0 = b * 1024 + q2 * 512
                pts = {}
                pot = {}
                if prev is not None:
                    pb, pq2, ppts = prev
                    pot0 = ot_pool.tile([65, 512], F32, tag="ot")
                    pot1 = ot_pool.tile([65, 512], F32, tag="ot")
                    pot = {0: pot0, 1: pot1}
                for kb in range(8):
                    k0 = b * 1024 + kb * 128
                    if prev is not None:
                        pb, pq2, ppts = prev
                        for h in (0, 1):
                            v2c = (pb * 8 + kb) * V2S + h * (HD + 1)
                            nc.tensor.matmul(
                                pot[h][:],
                                v2[:, v2c : v2c + HD + 1],
                                ppts[(h, kb)][:],
                                start=(kb == 0),
                                stop=(kb == 7),
                            )
                    for h in (0, 1):
                        hp = slice(h * 64, (h + 1) * 64)
                        stt = st_pool.tile([128, 512], F32, tag="st")
                        nc.tensor.matmul(
                            stt[:],
                            kt[hp, k0 : k0 + 128],
                            qt[hp, q0 : q0 + 512],
                            start=True,
                            stop=True,
                        )
                        pt = ptp.tile([128, 512], BF16)
                        nc.scalar.activation(pt[:], stt[:], EXP, scale=0.125)
                        pts[(h, kb)] = pt
                return pts, pot

            def s2_recip(state_ot):
                state = {}
                for h in (0, 1):
                    ot = state_ot[h]
                    sd = rcp.tile([65, 512], I32, tag="sd")
                    nc.vector.tensor_scalar(
                        sd[64:65, :],
                        ot[64:65, :].bitcast(I32),
                        -1,
                        RECIP_MAGIC,
                        mybir.AluOpType.mult,
                        mybir.AluOpType.add,
                    )
                    y0 = sd[64:65, :].bitcast(F32)
                    t1 = rcp.tile([65, 512], F32, tag="t1")
                    nc.vector.tensor_mul(t1[64:65, :], ot[64:65, :], y0)
                    m1 = rcp.tile([65, 512], F32, tag="m1")  # m1 = -y1
                    nc.vector.scalar_tensor_tensor(
                        m1[64:65, :],
                        t1[64:65, :],
                        2.0,
                        y0,
                        mybir.AluOpType.subtract,
                        mybir.AluOpType.mult,
                    )
                    t2 = rcp.tile([65, 512], F32, tag="t2")  # t2 = -s*y1
                    nc.vector.tensor_mul(t2[64:65, :], ot[64:65, :], m1[64:65, :])
                    m2 = rcp.tile([65, 512], F32R, tag="m2")  # m2 = -y2
                    nc.vector.scalar_tensor_tensor(
                        m2[64:65, :],
                        t2[64:65, :],
                        2.0,
                        m1[64:65, :],
                        mybir.AluOpType.add,
                        mybir.AluOpType.mult,
                    )
                    state[h] = (ot, m2)
                return state

            def s3_norm(b, q2, state):
                q0 = b * 1024 + q2 * 512
                for h in (0, 1):
                    ot, m2 = state[h]
                    bcp = po_pool.tile([64, 512], F32, tag="po")
                    nc.tensor.matmul(
                        bcp[:],
                        onef[64:65, 0:64],
                        m2[64:65, :],
                        start=True,
                        stop=True,
                    )
                    bc = bcs_pool.tile([64, 512], F32)
                    nc.scalar.copy(bc[:], bcp[:])
                    at = atall if h == 0 else at1
                    nc.vector.tensor_mul(at[0:64, q0 : q0 + 512], ot[0:64, :], bc[:])
                    if h == 1:
                        nc.sync.dma_start(
                            atall[64:128, q0 : q0 + 512], at1[0:64, q0 : q0 + 512]
                        )

            def cproj_stage(b):
                for oc in range(8):
                    ob = obp.tile([128, 1024], F16)
                    for q2 in range(2):
                        t0 = b * 1024 + q2 * 512
                        opt = po_pool.tile([128, 512], F32, tag="po")
                        nc.tensor.matmul(
                            opt[:],
                            wp_sb[:, oc * 128 : (oc + 1) * 128],
                            atall[:, t0 : t0 + 512],
                            start=True,
                            stop=True,
                        )
                        if oc % 4 == 3:
                            nc.scalar.copy(ob[:, q2 * 512 : (q2 + 1) * 512], opt[:])
                        else:
                            nc.vector.tensor_copy(
                                ob[:, q2 * 512 : (q2 + 1) * 512], opt[:]
                            )
                    nc.sync.dma_start(
                        out[oc * 128 : (oc + 1) * 128, b * 1024 : (b + 1) * 1024],
                        ob[:],
                    )

            groups = [(b, q2) for b in range(4) for q2 in range(2)]
            prev = None  # (b, q2, pts)
            s2_out = {}
            for i, (b, q2) in enumerate(groups):
                pts, pot = s1s2_interleaved(b, q2, prev)
                if prev is not None:
                    pb, pq2, _ = prev
                    s2_out[(pb, pq2)] = s2_recip(pot)
                if i >= 2:
                    qb, qq2 = groups[i - 2]
                    s3_norm(qb, qq2, s2_out.pop((qb, qq2)))
                    if qq2 == 1:
                        cproj_stage(qb)
                prev = (b, q2, pts)
            # drain: OT+recip for the last group, then the last two norms
            pb, pq2, ppts = prev
            pot0 = ot_pool.tile([65, 512], F32, tag="ot")
            pot1 = ot_pool.tile([65, 512], F32, tag="ot")
            pot = {0: pot0, 1: pot1}
            for kb in range(8):
                for h in (0, 1):
                    v2c = (pb * 8 + kb) * V2S + h * (HD + 1)
                    nc.tensor.matmul(
                        pot[h][:],
                        v2[:, v2c : v2c + HD + 1],
                        ppts[(h, kb)][:],
                        start=(kb == 0),
                        stop=(kb == 7),
                    )
            s2_out[(pb, pq2)] = s2_recip(pot)
            for (qb, qq2) in groups[-2:]:
                s3_norm(qb, qq2, s2_out.pop((qb, qq2)))
                if qq2 == 1:
                    cproj_stage(qb)


def _build_nc():
    nc = bacc.Bacc(
        "TRN2",
        target_bir_lowering=False,
        debug=False,
        enable_asserts=False,
        num_devices=NCORES,
    )
    xt = nc.dram_tensor("xt", [NX, T], BF16, kind="ExternalInput").ap()
    wq = nc.dram_tensor("wq", [128, 1024], BF16, kind="ExternalInput").ap()
    wk = nc.dram_tensor("wk", [128, 1024], BF16, kind="ExternalInput").ap()
    wv = nc.dram_tensor("wv", [128, 1024], BF16, kind="ExternalInput").ap()
    wp = nc.dram_tensor("wp", [128, 1024], BF16, kind="ExternalInput").ap()
    bq = nc.dram_tensor("bq", [128, 1], F32, kind="ExternalInput").ap()
    bk = nc.dram_tensor("bk", [128, 1], F32, kind="ExternalInput").ap()
    bv = nc.dram_tensor("bv", [128, 1], F32, kind="ExternalInput").ap()
    identd = nc.dram_tensor("ident", [128, 128], BF16, kind="ExternalInput").ap()
    onecd = nc.dram_tensor("onec", [128, 64], BF16, kind="ExternalInput").ap()
    onefd = nc.dram_tensor("onef", [128, 64], F32R, kind="ExternalInput").ap()
    out = nc.dram_tensor("out_t", [NX, T], F16, kind="ExternalOutput").ap()
    with tile.TileContext(nc) as tc:
        _emit(nc, tc, xt, wq, wk, wv, wp, bq, bk, bv, identd, onecd, onefd, out)
    nc.compile()
    return nc


def _pack_w(wcols):
    # [1024, 128] -> [128, 8*128] bf16: sbuf[p, k*128 + j] = W[k*128 + p, j]
    w = np.ascontiguousarray(np.asarray(wcols, dtype=np.float32))
    return np.ascontiguousarray(
        w.reshape(8, 128, 128).transpose(1, 0, 2).reshape(128, 1024)
    ).astype(ml_dtypes.bfloat16)


def kernel(hidden_states, c_attn_w, c_attn_b, c_proj_w, c_proj_b):
    global _nc_cache
    hidden_states = np.asarray(hidden_states, dtype=np.float32)
    c_attn_w = np.asarray(c_attn_w, dtype=np.float32)
    c_attn_b = np.asarray(c_attn_b, dtype=np.float32)
    c_proj_w = np.asarray(c_proj_w, dtype=np.float32)
    c_proj_b = np.asarray(c_proj_b, dtype=np.float32)

    if _nc_cache is None:
        _nc_cache = _build_nc()
    nc = _nc_cache

    X = hidden_states.reshape(T, NX)
    xt_np = np.ascontiguousarray(X.T).astype(ml_dtypes.bfloat16)

    in_maps = []
    for c in range(NCORES):
        cs = slice(c * 128, (c + 1) * 128)
        in_maps.append(
            {
                "xt": xt_np,
                "wq": _pack_w(c_attn_w[:, c * 128 : (c + 1) * 128]),
                "wk": _pack_w(c_attn_w[:, 1024 + c * 128 : 1024 + (c + 1) * 128]),
                "wv": _pack_w(c_attn_w[:, 2048 + c * 128 : 2048 + (c + 1) * 128]),
                "wp": np.ascontiguousarray(c_proj_w[cs, :]).astype(ml_dtypes.bfloat16),
                "bq": np.ascontiguousarray(c_attn_b[cs].reshape(128, 1)),
                "bk": np.ascontiguousarray(
                    c_attn_b[1024 + c * 128 : 1024 + (c + 1) * 128].reshape(128, 1)
                ),
                "bv": np.ascontiguousarray(
                    c_attn_b[2048 + c * 128 : 2048 + (c + 1) * 128].reshape(128, 1)
                ),
                "ident": np.eye(128, dtype=np.float32).astype(ml_dtypes.bfloat16),
                "onec": np.ones((128, 64), dtype=ml_dtypes.bfloat16),
                "onef": np.full((128, 64), -1.0, dtype=np.float32),
            }
        )

    trace = bool(int(os.environ.get("KERNEL_PROFILE", "0")))
    if trace:
        trace = _ensure_ntff_hook()
    try:
        res = run_bass_kernel_spmd(
            nc, in_maps, core_ids=list(range(NCORES)), trace=trace
        )
    except Exception:
        if not trace:
            raise
        print("kernel.py: traced run failed; retrying untraced")
        res = run_bass_kernel_spmd(nc, in_maps, core_ids=list(range(NCORES)))

    total = np.zeros((NX, T), np.float32)
    for r in res.results:
        total += r["out_t"].astype(np.float32)
    out = total.T.reshape(B, S, NX) + c_proj_b[None, None, :]
    kernel.last_exec_time_ns = res.exec_time_ns
    return out.astype(np.float32)
